# revision 1
# baseline (speedup 1.0000x reference)
"""Distributed Trainium2 Bass kernel for nn_Attention_57346403336225.

Reference computation (per batch b of 16, N=1024 tokens, E=128 emb, H=8 heads,
head dim d = E = 128, INNER = 1024):
    xn   = LayerNorm(x) * ln_w + ln_b
    qkv  = xn @ w_qkv ; q,k,v heads
    dots = (q @ k^T) * scale[h]  ; diagonal masked to -FLT_MAX
    attn = softmax(dots) ; out = attn @ v
    y    = out @ w_proj + b_proj

Sharding: pure data-parallel over batch (16 / 8 cores = 2 batches per core),
weights replicated, no collectives.

Per-core algorithm (all heads/batches looped on-chip):
  - host precomputes A_h = scale[h] * Wq_h @ Wk_h^T  [E,E]  so that
    dots_h = xn @ A_h @ xn^T  (one [E,E] matmul replaces separate q,k)
  - LayerNorm in [tok, E] layout, PE-transpose -> xnT [E, tok].  All Sqrt
    activations run in the prologue (batch-0 stats via DVE bn_stats,
    batch-1 stats via ScalarE Copy/Square accumulation passes) so the
    ScalarE activation table never swaps between Sqrt and Exp mid-loop
    (each swap costs ~1.3us).  Batch-1 normalize/transpose/v-projection
    are spread one token-tile per attention iteration.
  - dots tiles [128q, 1024k] in PSUM (bf16 matmuls), diagonal mask added on
    DVE, exp on ScalarE with fused per-row accumulation (softmax denom).
    The exp output is written key-tile-major into P so that the whole
    4-query-tile block is contiguous for the xbar transpose AND the
    transposed strips come out contiguous per key tile for PV.
  - P^T via DMA xbar transposes batched 4 query-tiles per call (the xbar
    launch overhead dominates small calls).
  - out^T accumulation over k chunks (bf16); projection per head with the
    softmax normalization (1/rowsum) applied as a per-partition scale in
    the projection epilogue, accumulated over heads on DVE. proj is emitted
    with pushed-back scheduler priority so its small matmuls don't block
    the next iteration's dots matmuls at the iteration boundary.
  - Sync engine owns only the xbar transposes; x loads / output stores go
    through the gpsimd SWDGE queue; weight loads on gpsimd in the prologue.
"""

import numpy as np
import ml_dtypes

B, N, E, H = 16, 1024, 128, 8
NCORES = 8
B_LOC = B // NCORES  # 2
LN_EPS = 1e-5
NT = N // 128    # 8 token tiles per batch
MASK_VAL = -1e30

_cache = {}


def _build_nc():
    import concourse.bacc as bacc
    import concourse.mybir as mybir
    import concourse.tile as tile

    f32 = mybir.dt.float32
    bf16 = mybir.dt.bfloat16
    Exp = mybir.ActivationFunctionType.Exp
    Sqrt = mybir.ActivationFunctionType.Sqrt
    Square = mybir.ActivationFunctionType.Square
    Copy = mybir.ActivationFunctionType.Copy
    sub = mybir.AluOpType.subtract
    mult = mybir.AluOpType.mult
    add = mybir.AluOpType.add

    nc = bacc.Bacc("TRN2", target_bir_lowering=False)

    x_p = nc.declare_dram_parameter("x", [B_LOC, N, E], f32, isOutput=False)
    a_p = nc.declare_dram_parameter("amat", [H, E, E], bf16, isOutput=False)
    wvf_p = nc.declare_dram_parameter("wvf", [E, H * E], bf16, isOutput=False)
    wp_p = nc.declare_dram_parameter("wp", [H, E, E], bf16, isOutput=False)
    lnw_p = nc.declare_dram_parameter("lnw", [E, 1], f32, isOutput=False)
    lnb_p = nc.declare_dram_parameter("lnb", [E, 1], f32, isOutput=False)
    id_p = nc.declare_dram_parameter("ident", [E, E], f32, isOutput=False)
    dm_p = nc.declare_dram_parameter("dmask", [E, E], f32, isOutput=False)
    bp_p = nc.declare_dram_parameter("bptile", [128, E], f32, isOutput=False)
    out_p = nc.declare_dram_parameter("out", [B_LOC, N, E], f32, isOutput=True)

    with tile.TileContext(nc) as tc:
        with (
            tc.tile_pool(name="const", bufs=1) as cpool,
            tc.tile_pool(name="ln", bufs=8) as lnpool,
            tc.tile_pool(name="work", bufs=4) as wpool,
            tc.tile_pool(name="bigP", bufs=2) as ppool,
            tc.tile_pool(name="bigPT", bufs=4) as ptpool,
            tc.tile_pool(name="psd", bufs=3, space="PSUM") as psd,
            tc.tile_pool(name="psm", bufs=2, space="PSUM") as psm,
        ):
            # ---- constants ----
            ident = cpool.tile([E, E], f32, tag="ident")
            dmask = cpool.tile([E, E], f32, tag="dmask")
            lnw = cpool.tile([E, 1], f32, tag="lnw")
            lnb = cpool.tile([E, 1], f32, tag="lnb")
            amat = cpool.tile([E, H, E], bf16, tag="amat")
            wvf = cpool.tile([E, H * E], bf16, tag="wvf")
            wp = cpool.tile([E, H, E], bf16, tag="wp")
            bptile = cpool.tile([128, E], f32, tag="bptile")
            epst = cpool.tile([128, 1], f32, tag="epst")
            zbias = cpool.tile([128, 1], f32, tag="zbias")
            scr = cpool.tile([128, E], f32, tag="scr")  # stats scratch
            xall = [cpool.tile([128, NT, E], f32, tag=f"xall{b}",
                               name=f"xall{b}") for b in range(B_LOC)]
            xnT = [cpool.tile([E, N], bf16, tag=f"xnT{b}", name=f"xnT{b}")
                   for b in range(B_LOC)]
            vall = [cpool.tile([128, NT, H * E], bf16, tag=f"vall{b}",
                               name=f"vall{b}") for b in range(B_LOC)]
            y_acc = [cpool.tile([128, N], f32, tag=f"yacc{b}", name=f"yacc{b}")
                     for b in range(B_LOC)]
            # per-token LN coefficients: xn = x * rs - mur
            rs8 = [cpool.tile([128, NT], f32, tag=f"rs8{b}", name=f"rs8{b}")
                   for b in range(B_LOC)]
            mur8 = [cpool.tile([128, NT], f32, tag=f"mur8{b}",
                               name=f"mur8{b}") for b in range(B_LOC)]

            # batch-0 x tiles first so LayerNorm can start right away
            for t in range(NT):
                nc.sync.dma_start(xall[0][:, t, :],
                                  x_p[0, t * 128:(t + 1) * 128, :])
            nc.sync.dma_start(ident[:], id_p[:])
            nc.sync.dma_start(lnw[:], lnw_p[:])
            nc.sync.dma_start(lnb[:], lnb_p[:])
            # batch-1 x before the weights: ScalarE stat passes need it early
            for t in range(NT):
                nc.gpsimd.dma_start(xall[1][:, t, :],
                                    x_p[1, t * 128:(t + 1) * 128, :])
            nc.gpsimd.dma_start(amat[:], a_p[:].rearrange("h a b -> a h b"))
            nc.gpsimd.dma_start(wvf[:], wvf_p[:])
            nc.gpsimd.dma_start(wp[:], wp_p[:].rearrange("h a b -> a h b"))
            nc.gpsimd.dma_start(dmask[:], dm_p[:])
            nc.gpsimd.dma_start(bptile[:], bp_p[:])
            nc.vector.memset(epst[:], LN_EPS)
            nc.vector.memset(zbias[:], 0.0)

            def cast_psum(eng, out, in_):
                """PSUM f32 -> SBUF bf16 cast."""
                eng.tensor_copy(out, in_)

            # ---- batch-0 LN stats on DVE (stage-major), one batched Sqrt --
            mvall0 = cpool.tile([128, NT, 2], f32, tag="mvall0")
            stl = []
            for t in range(NT):
                st = lnpool.tile([128, 6], f32, tag="st", name=f"st0_{t}")
                nc.vector.bn_stats(st[:], xall[0][:, t, :])
                stl.append(st)
            for t in range(NT):
                nc.vector.bn_aggr(mvall0[:, t, :], stl[t][:])
            sd0 = cpool.tile([128, NT], f32, tag="sd0")
            nc.scalar.activation(sd0[:], mvall0[:, :, 1], Sqrt, bias=epst[:])
            nc.vector.reciprocal(rs8[0][:], sd0[:])
            nc.vector.tensor_tensor(mur8[0][:], mvall0[:, :, 0], rs8[0][:],
                                    op=mult)

            # ---- batch-1 LN stats on ScalarE (Copy/Square accum passes) ----
            sx1 = cpool.tile([128, NT], f32, tag="sx1")
            sq1 = cpool.tile([128, NT], f32, tag="sq1")
            for t in range(NT):
                nc.scalar.activation(scr[:], xall[1][:, t, :], Copy,
                                     accum_out=sx1[:, t:t + 1])
                nc.scalar.activation(scr[:], xall[1][:, t, :], Square,
                                     accum_out=sq1[:, t:t + 1])
            mu1 = cpool.tile([128, NT], f32, tag="mu1")
            nc.vector.tensor_scalar(mu1[:], sx1[:], 1.0 / E, None, op0=mult)
            m21 = cpool.tile([128, NT], f32, tag="m21")
            nc.vector.tensor_tensor(m21[:], sx1[:], mu1[:], op=mult)
            v128 = cpool.tile([128, NT], f32, tag="v128")
            nc.vector.tensor_tensor(v128[:], sq1[:], m21[:], op=sub)
            sd1 = cpool.tile([128, NT], f32, tag="sd1")
            nc.scalar.activation(sd1[:], v128[:], Sqrt, bias=epst[:],
                                 scale=1.0 / E)
            nc.vector.reciprocal(rs8[1][:], sd1[:])
            nc.vector.tensor_tensor(mur8[1][:], mu1[:], rs8[1][:], op=mult)

            def ln_apply(b, t):
                """xn = x*rs - mur, PE transpose, ln_w/ln_b epilogue."""
                xn = lnpool.tile([128, E], f32, tag="xnt", name=f"xn{b}_{t}")
                nc.vector.tensor_scalar(
                    xn[:], xall[b][:, t, :], rs8[b][:, t:t + 1],
                    mur8[b][:, t:t + 1], op0=mult, op1=sub,
                )
                tp = psm.tile([128, E], f32, tag="m512", name=f"lntp{b}_{t}")
                nc.tensor.transpose(tp[:], xn[:], ident[:])
                nc.vector.tensor_scalar(
                    xnT[b][:, t * 128:(t + 1) * 128], tp[:],
                    lnw[:], lnb[:], op0=mult, op1=add,
                )

            def v_proj(b, ts_list):
                """v = xn @ Wv for token tiles (all heads); casts on DVE."""
                for t in ts_list:
                    for c in range(2):
                        vps = psm.tile([128, 512], f32, tag="m512",
                                       name=f"vps{b}_{t}_{c}")
                        nc.tensor.matmul(
                            vps[:],
                            xnT[b][:, t * 128:(t + 1) * 128],
                            wvf[:, c * 512:(c + 1) * 512],
                            start=True, stop=True,
                        )
                        cast_psum(nc.vector,
                                  vall[b][:, t, c * 512:(c + 1) * 512],
                                  vps[:])

            for t in range(NT):
                ln_apply(0, t)

            # ---- attention, software-pipelined across (batch, head) ----
            iters = [(b, h) for b in range(B_LOC) for h in range(H)]
            NIT = len(iters)
            stash = {}

            def make_tT(it):
                b, h = iters[it]
                tT = wpool.tile([E, N], bf16, tag="tT", name=f"tT{it}")
                for qc in range(2):
                    tps = psm.tile([128, 512], f32, tag="m512",
                                   name=f"tps{it}_{qc}")
                    nc.tensor.matmul(
                        tps[:], amat[:, h, :],
                        xnT[b][:, qc * 512:(qc + 1) * 512],
                        start=True, stop=True,
                    )
                    cast_psum(nc.vector, tT[:, qc * 512:(qc + 1) * 512],
                              tps[:])
                stash[("tT", it)] = tT

            make_tT(0)
            with tc.high_priority(offset=-90):
                v_proj(0, list(range(NT)))

            def dots_group(it, g, P, rsum):
                """Query tiles qt in [4g, 4g+4): dots matmuls, diag mask,
                exp writing P key-tile-major: within group g the exp output
                for query tile a=qt%4 lands at positions kt*512 + a*128 + c,
                so P[:, g, :] is one contiguous [128, 4096] transpose source
                and the transposed strips are contiguous per key tile."""
                b, h = iters[it]
                tT = stash[("tT", it)]
                Pg = P[:, g, :].rearrange("p (kt a c) -> p kt a c", a=4, c=128)
                for qt in range(4 * g, 4 * g + 4):
                    dps = psd.tile([128, N], f32, tag="dots",
                                   name=f"dps{it}_{qt}")
                    for kc in range(2):
                        nc.tensor.matmul(
                            dps[:, kc * 512:(kc + 1) * 512],
                            tT[:, qt * 128:(qt + 1) * 128],
                            xnT[b][:, kc * 512:(kc + 1) * 512],
                            start=True, stop=True,
                        )
                    nc.vector.tensor_add(
                        dps[:, qt * 128:(qt + 1) * 128],
                        dps[:, qt * 128:(qt + 1) * 128],
                        dmask[:],
                    )
                    nc.scalar.activation(
                        Pg[:, :, qt % 4, :],
                        dps[:].rearrange("p (kt c) -> p kt c", c=128),
                        Exp, bias=zbias[:],
                        accum_out=rsum[:, qt:qt + 1],
                    )
                PTc = ptpool.tile([128, 4 * NT, 128], bf16, tag="PT",
                                  name=f"PTc{it}_{g}")
                nc.sync.dma_start(
                    out=PTc[:],
                    in_=P[:, g, :],
                    transpose=True,
                )
                stash[("PT", it, g)] = PTc

            def dots_group_ilv(it, g, P, rsum, pv_it):
                """dots_group(it, g) with the PV chunk-0 matmuls of iter
                pv_it interleaved two-per-query-tile (different PSUM banks
                between consecutive same-bank accumulates)."""
                b, h = iters[it]
                tT = stash[("tT", it)]
                Pg = P[:, g, :].rearrange("p (kt a c) -> p kt a c", a=4, c=128)
                if pv_it is not None:
                    pv_start(pv_it, 0)
                for idx, qt in enumerate(range(4 * g, 4 * g + 4)):
                    dps = psd.tile([128, N], f32, tag="dots",
                                   name=f"dps{it}_{qt}")
                    for kc in range(2):
                        nc.tensor.matmul(
                            dps[:, kc * 512:(kc + 1) * 512],
                            tT[:, qt * 128:(qt + 1) * 128],
                            xnT[b][:, kc * 512:(kc + 1) * 512],
                            start=True, stop=True,
                        )
                    if pv_it is not None:
                        pv_mm(pv_it, 0, 2 * idx)
                        pv_mm(pv_it, 0, 2 * idx + 1)
                    nc.vector.tensor_add(
                        dps[:, qt * 128:(qt + 1) * 128],
                        dps[:, qt * 128:(qt + 1) * 128],
                        dmask[:],
                    )
                    nc.scalar.activation(
                        Pg[:, :, qt % 4, :],
                        dps[:].rearrange("p (kt c) -> p kt c", c=128),
                        Exp, bias=zbias[:],
                        accum_out=rsum[:, qt:qt + 1],
                    )
                if pv_it is not None:
                    pv_finish(pv_it, 0)
                PTc = ptpool.tile([128, 4 * NT, 128], bf16, tag="PT",
                                  name=f"PTc{it}_{g}")
                nc.sync.dma_start(
                    out=PTc[:],
                    in_=P[:, g, :],
                    transpose=True,
                )
                stash[("PT", it, g)] = PTc

            def pv_start(it, qc):
                """Allocate the PSUM accumulator for PV chunk qc of iter it.
                The 8 chain matmuls are emitted via pv_mm (interleaved with
                other-bank matmuls to dodge the same-bank accumulate RMW
                bubble: back-to-back accumulates to one bank run at 427 ns
                instead of 216)."""
                ops = psm.tile([128, 512], f32, tag="m512",
                               name=f"ops{it}_{qc}")
                stash[("ops", it, qc)] = ops

            def pv_mm(it, qc, kt):
                b, h = iters[it]
                PTc = stash[("PT", it, qc)]
                ops = stash[("ops", it, qc)]
                nc.tensor.matmul(
                    ops[:],
                    vall[b][:, kt, h * E:(h + 1) * E],
                    PTc[:, 4 * kt:4 * (kt + 1), :],
                    start=(kt == 0), stop=(kt == NT - 1),
                )

            def pv_finish(it, qc):
                oT = stash[("oT", it)]
                ops = stash.pop(("ops", it, qc))
                cast_psum(nc.vector, oT[:, qc * 512:(qc + 1) * 512], ops[:])

            def pv_chain(it, qc):
                pv_start(it, qc)
                for kt in range(NT):
                    pv_mm(it, qc, kt)
                pv_finish(it, qc)

            def proj_mm(it, t):
                b, h = iters[it]
                oT = stash[("oT", it)]
                yps = psm.tile([128, E], f32, tag="m512",
                               name=f"yps{it}_{t}")
                nc.tensor.matmul(
                    yps[:],
                    oT[:, t * 128:(t + 1) * 128],
                    wp[:, h, :],
                    start=True, stop=True,
                )
                stash[("yps", it, t)] = yps

            def proj_ep(it, t):
                b, h = iters[it]
                rcp = stash[("rcp", it)]
                yps = stash.pop(("yps", it, t))
                if h == 0:
                    nc.vector.scalar_tensor_tensor(
                        y_acc[b][:, t * 128:(t + 1) * 128],
                        yps[:], rcp[:, t:t + 1], bptile[:],
                        op0=mult, op1=add,
                    )
                else:
                    nc.vector.scalar_tensor_tensor(
                        y_acc[b][:, t * 128:(t + 1) * 128],
                        yps[:], rcp[:, t:t + 1],
                        y_acc[b][:, t * 128:(t + 1) * 128],
                        op0=mult, op1=add,
                    )
                if h == H - 1 and t % 2 == 1:
                    nc.gpsimd.dma_start(
                        out_p[b, (t - 1) * 128:(t + 1) * 128].rearrange(
                            "(u p) e -> p u e", p=128),
                        y_acc[b][:, (t - 1) * 128:(t + 1) * 128].rearrange(
                            "p (u e) -> p u e", u=2),
                    )

            def proj_drop(it):
                stash.pop(("oT", it))
                stash.pop(("rcp", it))
                stash.pop(("tT", it))
                stash.pop(("PT", it, 0))
                stash.pop(("PT", it, 1))

            def pv_proj_ilv(pv_it, proj_it):
                """PV chunk-1 matmuls of iter pv_it interleaved 1:1 with the
                projection matmuls of iter proj_it (different PSUM pools, so
                consecutive same-bank accumulates are spaced out), followed
                by the projection epilogue."""
                if pv_it is not None:
                    pv_start(pv_it, 1)
                for j in range(NT):
                    if pv_it is not None:
                        pv_mm(pv_it, 1, j)
                    if proj_it is not None:
                        proj_mm(proj_it, j)
                        if j >= 1:
                            proj_ep(proj_it, j - 1)
                if pv_it is not None:
                    pv_finish(pv_it, 1)
                if proj_it is not None:
                    proj_ep(proj_it, NT - 1)
                    proj_drop(proj_it)

            # batch-1 LN apply spread one tile per iteration; xnT[1] needed
            # by make_tT(8) (emitted at it=7), vall[1] by pv(8) at it=9.
            for it in range(NIT + 2):
                cur = it if it < NIT else None
                # batch-1 LN apply FIRST so the xnT[1] write of tile `it`
                # precedes make_tT(it+1)'s read in program order (tile-level
                # dependency tracking would otherwise order the write after
                # the read and make_tT(8) would see stale data)
                if it < NT:
                    ln_apply(1, it)
                    v_proj(1, [it])
                if cur is not None:
                    b, h = iters[cur]
                    P = ppool.tile([128, 2, 4 * N], bf16, tag="P",
                                   name=f"P{cur}")
                    rsum = wpool.tile([128, NT], f32, tag="rsum",
                                      name=f"rsum{cur}")
                    oT = wpool.tile([E, N], bf16, tag="oT", name=f"oT{cur}")
                    stash[("oT", cur)] = oT
                    dots_group(cur, 0, P, rsum)
                if cur is not None and cur + 1 < NIT:
                    make_tT(cur + 1)
                if 0 <= it - 1 < NIT and it - 1 != NIT - 1:
                    pv_chain(it - 1, 0)
                if cur is not None:
                    dots_group(cur, 1, P, rsum)
                if 0 <= it - 1 < NIT:
                    pv_chain(it - 1, 1)
                if cur is not None:
                    rcp = wpool.tile([128, NT], f32, tag="rcp",
                                     name=f"rcp{cur}")
                    nc.vector.reciprocal(rcp[:], rsum[:])
                    stash[("rcp", cur)] = rcp
                if cur is not None and cur == NIT - 1:
                    # drain: start the last PV first-half early (its
                    # transpose lands mid-iteration)
                    pv_chain(cur, 0)
                if 0 <= it - 2 < NIT:
                    with tc.high_priority(offset=-60):
                        for t in range(NT):
                            proj_mm(it - 2, t)
                        for t in range(NT):
                            proj_ep(it - 2, t)
                        proj_drop(it - 2)

    nc.compile()
    return nc


def _get_nc():
    if "nc" not in _cache:
        _cache["nc"] = _build_nc()
    return _cache["nc"]


def _make_in_maps(inputs):
    x = np.ascontiguousarray(np.asarray(inputs["x"], dtype=np.float32))
    ln_w = np.asarray(inputs["ln_w"], dtype=np.float32)
    ln_b = np.asarray(inputs["ln_b"], dtype=np.float32)
    w_qkv = np.asarray(inputs["w_qkv"], dtype=np.float32)
    scale = np.asarray(inputs["scale"], dtype=np.float32)
    w_proj = np.asarray(inputs["w_proj"], dtype=np.float32)

    INNER = E * H
    Wq = w_qkv[:, :INNER]
    Wk = w_qkv[:, INNER:2 * INNER]
    Wv = w_qkv[:, 2 * INNER:]

    amat = np.stack(
        [scale[h] * (Wq[:, h * E:(h + 1) * E] @ Wk[:, h * E:(h + 1) * E].T)
         for h in range(H)]
    ).astype(ml_dtypes.bfloat16)  # [H, E, E]
    wvf = Wv.astype(ml_dtypes.bfloat16)  # [E, INNER]
    wp = w_proj.reshape(H, E, E).astype(ml_dtypes.bfloat16)  # [H, d, E]
    ident = np.eye(E, dtype=np.float32)
    dmask = (np.eye(E, dtype=np.float32) * MASK_VAL).astype(np.float32)
    lnw = ln_w.reshape(E, 1)
    lnb = ln_b.reshape(E, 1)

    b_proj = np.asarray(inputs["b_proj"], dtype=np.float32)
    bptile = np.broadcast_to(b_proj[None, :], (128, E)).copy()
    shared = {
        "amat": amat, "wvf": wvf, "wp": wp, "bptile": bptile,
        "lnw": lnw, "lnb": lnb, "ident": ident, "dmask": dmask,
    }
    return [
        {"x": x[c * B_LOC:(c + 1) * B_LOC], **shared} for c in range(NCORES)
    ]


def kernel(x, ln_w, ln_b, w_qkv, scale, w_proj, b_proj):
    from concourse.bass_utils import run_bass_kernel_spmd

    in_maps = _make_in_maps(dict(
        x=x, ln_w=ln_w, ln_b=ln_b, w_qkv=w_qkv, scale=scale,
        w_proj=w_proj, b_proj=b_proj,
    ))

    nc = _get_nc()
    res = run_bass_kernel_spmd(nc, in_maps, core_ids=list(range(NCORES)))
    y = np.concatenate([res.results[c]["out"] for c in range(NCORES)], axis=0)
    return y.astype(np.float32)

